# revision 3
# baseline (speedup 1.0000x reference)
"""LBP-5x3 code kernel for TRN2 (8 NeuronCores, data parallel) + host binning.

Full inputs: x [128, 512, 512] fp32 in [0,1). Output: [128, 59] fp32.
Each core processes 16 images laid out [128 partitions = rows-in-block,
4 row-blocks x (5+512+5) haloed cols].

The 8 LBP neighbors form 4 symmetric offset pairs +-Delta. For each pair the
device computes ONE non-strict mask m(P) = [im(P+Delta) >= im(P)] (DVE is_ge;
4 passes instead of 8). The backward bit is recovered linearly:
  m_bwd(P) = [im(P-Delta) >= im(P)] = 1 - m(P-Delta) + [im(P-Delta)==im(P)]
The PE reads each mask tile twice - once aligned with weight +w_fwd, once
shifted by Delta with weight -w_bwd (row shift folded into a shifted-identity
lhsT, col shift into the rhs column offset) - and the PSUM->SBUF evacuation
adds the constant sum(w_bwd)=195. The tie term and all border rows/cols are
patched exactly on the host (~1-2% of pixels), as are RNE-vs-floor pixels of
the device's uint8 quantization.

Per image on device:
  ACT:  ci = RNE(x*255 - 0.5) -> int16, haloed cols (halos zeroed, Pool)
  DMA:  up3[p] = ci[p+3] (one SBUF-SBUF row-shift + block wrap)
  DVE:  4 is_ge masks -> bf16, haloed mask tiles (halos zeroed once)
  PE:   7 matmuls per 512-col chunk (3 pairs fwd+bwd, dx=0 pair fused)
  ACT:  PSUM + 195 -> uint8 codes, DMA out
Host: patch rows {128b+0..2, 509..511}, cols {0..4, 509..511}, tie pixels,
RNE pixels; per-image 256-bincount -> 58 uniform bins + catch-all, mod 256.
"""
import sys

sys.path.insert(0, "/opt/trn_rl_repo")
sys.path.insert(0, "/opt/pypackages")

import numpy as np
import ml_dtypes

import concourse.bacc as bacc
import concourse.tile as tile
from concourse import mybir
from concourse.bass_utils import run_bass_kernel_spmd

UNIS = np.array([0, 1, 2, 3, 4, 6, 7, 8, 12, 14, 15, 16, 24, 28, 30, 31, 32, 48, 56,
                 60, 62, 63, 64, 96, 112, 120, 124, 126, 127, 128, 129, 131, 135, 143,
                 159, 191, 192, 193, 195, 199, 207, 223, 224, 225, 227, 231, 239, 240,
                 241, 243, 247, 248, 249, 251, 252, 253, 254, 255], dtype=np.int32)

# 4 symmetric pairs: (dy, dx) forward offset, forward weight, backward weight
PAIRS = [(3, 3, 8, 128), (3, 0, 16, 1), (3, -3, 32, 2), (0, 5, 4, 64)]
WSUM_BWD = float(sum(wb for (_, _, _, wb) in PAIRS))  # 195

NIMG = 16          # images per core
H = W = 512
NB = 4             # row blocks of 128
OFFC = 5           # halo width (image col offset inside a block)
BW = W + 2 * OFFC  # block width with halo (522)
FW = NB * BW       # full free width of haloed tiles (2088)
CW = NB * W        # full free width of compact tiles (2048)

F32 = mybir.dt.float32
BF16 = mybir.dt.bfloat16
I16 = mybir.dt.int16
U8 = mybir.dt.uint8

_CACHE = {}


def _hv(t, start, width=W):
    """3D AP over a haloed [128, FW] tile: 4 blocks x width cols from `start`."""
    return t[:].rearrange("p (b c) -> p b c", b=NB)[:, :, start:start + width]


def _weight_mats() -> np.ndarray:
    """lhsT weight matrices [7, 128, 128] bf16: out[p,c] = sum_q W[q,p] m[q,c].
    Aligned read: W = w*I. Row-shifted read (mask lives at q = r-3):
    W[q, p] = w * delta(q == p-3). The dx=0 pair fuses fwd+bwd into one W."""
    eye = np.eye(128, dtype=np.float32)
    shift = np.zeros((128, 128), dtype=np.float32)
    shift[np.arange(125), np.arange(3, 128)] = 1.0  # S[q, p] = 1 iff q == p-3
    mats = [
        16.0 * eye - 1.0 * shift,   # pair (3,0): fused fwd 16I, bwd -1*S
        8.0 * eye,                  # pair (3,3) fwd
        -128.0 * shift,             # pair (3,3) bwd
        32.0 * eye,                 # pair (3,-3) fwd
        -2.0 * shift,               # pair (3,-3) bwd
        4.0 * eye,                  # pair (0,5) fwd
        -64.0 * eye,                # pair (0,5) bwd
    ]
    return np.stack(mats).astype(ml_dtypes.bfloat16)


def _build_nc():
    nc = bacc.Bacc("TRN2", target_bir_lowering=False, debug=False, num_devices=8)
    x = nc.dram_tensor("x", [NIMG, H, W], F32, kind="ExternalInput")
    wmat = nc.dram_tensor("wmat", [7, 128, 128], BF16, kind="ExternalInput")
    codes_dram = nc.dram_tensor("codes", [NIMG, H, W], U8, kind="ExternalOutput")

    with tile.TileContext(nc) as tc:
        with tc.tile_pool(name="pc", bufs=1) as poolc, \
                tc.tile_pool(name="px", bufs=5) as poolx, \
                tc.tile_pool(name="pm", bufs=2) as poolm, \
                tc.tile_pool(name="ps", bufs=8, space="PSUM") as poolp:
            wt = poolc.tile([128, 7 * 128], BF16, name="wt", tag="wt")
            # (wmat DMA is emitted after image 0's first block load)

            def w_ap(k):
                return wt[:, k * 128:(k + 1) * 128]

            for img in range(NIMG):
                xf = poolx.tile([128, CW], F32, name="xf", tag="xf")
                ci = poolx.tile([128, FW], I16, name="ci", tag="ci")
                nc.gpsimd.memset(ci[:].rearrange("p (b c) -> p b c", b=NB)[:, :, 0:OFFC], 0.0)
                nc.gpsimd.memset(ci[:].rearrange("p (b c) -> p b c", b=NB)[:, :, OFFC + W:BW], 0.0)
                if img == 0:
                    # block-granular prologue: lets the first compares (and so
                    # the PE) start ~3 DMA+convert legs earlier
                    for b in range(NB):
                        nc.sync.dma_start(
                            xf[:].rearrange("p (b c) -> p b c", b=NB)[:, b:b + 1, :],
                            x.ap()[img].rearrange("(b p) c -> p b c", b=NB)[:, b:b + 1, :])
                        if b == 0:
                            nc.sync.dma_start(wt[:].rearrange("p (k q) -> p k q", k=7),
                                              wmat.ap().rearrange("k p q -> p k q"))
                        nc.scalar.activation(
                            out=_hv(ci, OFFC)[:, b:b + 1, :],
                            in_=xf[:].rearrange("p (b c) -> p b c", b=NB)[:, b:b + 1, :],
                            func=mybir.ActivationFunctionType.Copy,
                            bias=-0.5, scale=255.0)
                else:
                    nc.sync.dma_start(xf[:].rearrange("p (b c) -> p b c", b=NB),
                                      x.ap()[img].rearrange("(b p) c -> p b c", b=NB))
                    nc.scalar.activation(out=_hv(ci, OFFC),
                                         in_=xf[:].rearrange("p (b c) -> p b c", b=NB),
                                         func=mybir.ActivationFunctionType.Copy,
                                         bias=-0.5, scale=255.0)
                # up3[p] = ci[p+3]: rows shifted up by 3 within each block,
                # block wrap for the last 3 partitions. The tail (rows past
                # 511) stays stale (finite int16); those rows are host-patched.
                up3 = poolx.tile([128, FW], I16, name="up3", tag="up3")
                if img == 0:
                    for b in range(NB):
                        nc.sync.dma_start(up3[0:125, b * BW:(b + 1) * BW],
                                          ci[3:128, b * BW:(b + 1) * BW])
                        if b > 0:
                            nc.sync.dma_start(up3[125:128, (b - 1) * BW:b * BW],
                                              ci[0:3, b * BW:(b + 1) * BW])
                else:
                    nc.sync.dma_start(up3[0:125, :], ci[3:128, :])
                    nc.sync.dma_start(up3[125:128, 0:FW - BW].rearrange("p (b c) -> p b c", b=NB - 1),
                                      ci[0:3, BW:FW].rearrange("p (b c) -> p b c", b=NB - 1))

                # 4 non-strict masks, haloed tiles (halos zeroed on the first
                # two iterations = both pool buffers, never written again).
                # The (0,5) mask goes first: it needs only ci, so the PE can
                # start before the up3 shift DMA lands.
                masks = {}
                for pi, (dy, dx, _, _) in sorted(enumerate(PAIRS),
                                                 key=lambda e: e[1][0]):
                    mt = poolm.tile([128, FW], BF16, name=f"m{pi}", tag=f"m{pi}")
                    if img < 2:
                        nc.gpsimd.memset(mt[:].rearrange("p (b c) -> p b c", b=NB)[:, :, 0:OFFC], 0.0)
                        nc.gpsimd.memset(mt[:].rearrange("p (b c) -> p b c", b=NB)[:, :, OFFC + W:BW], 0.0)
                    src = up3 if dy == 3 else ci
                    if img == 0:
                        # per-block, so each PE chunk can start off block ch
                        for b in range(NB):
                            nc.vector.tensor_tensor(
                                out=_hv(mt, OFFC)[:, b:b + 1, :],
                                in0=_hv(src, OFFC + dx)[:, b:b + 1, :],
                                in1=_hv(ci, OFFC)[:, b:b + 1, :],
                                op=mybir.AluOpType.is_ge)
                    else:
                        nc.vector.tensor_tensor(out=_hv(mt, OFFC),
                                                in0=_hv(src, OFFC + dx),
                                                in1=_hv(ci, OFFC),
                                                op=mybir.AluOpType.is_ge)
                    masks[pi] = mt

                # 7 matmuls per 512-col chunk: (lhsT index, mask, col offset),
                # ordered by mask readiness
                mm = [(5, masks[3], OFFC),        # (0,5) fwd +4I
                      (6, masks[3], OFFC - 5),    # (0,5) bwd -64I, cols c-5
                      (1, masks[0], OFFC),        # (3,3) fwd +8I
                      (2, masks[0], OFFC - 3),    # (3,3) bwd -128S, cols c-3
                      (0, masks[1], OFFC),        # (3,0) fused 16I - S
                      (3, masks[2], OFFC),        # (3,-3) fwd +32I
                      (4, masks[2], OFFC + 3)]    # (3,-3) bwd -2S, cols c+3
                code8 = poolx.tile([128, CW], U8, name="code8", tag="code8")
                for ch in range(NB):
                    cps = poolp.tile([128, W], F32, name="cps", tag="cps")
                    for i, (k, mt, co) in enumerate(mm):
                        rhs = mt[:, ch * BW + co: ch * BW + co + W]
                        nc.tensor.matmul(out=cps[:], lhsT=w_ap(k), rhs=rhs,
                                         start=(i == 0), stop=(i == len(mm) - 1))
                    # code = psum + sum(w_bwd); exact ints, safe in [0, 255]
                    nc.scalar.activation(out=code8[:, ch * W:(ch + 1) * W], in_=cps[:],
                                         func=mybir.ActivationFunctionType.Copy,
                                         bias=WSUM_BWD, scale=1.0)
                if img == NIMG - 1:
                    # drain faster: ship chunks as their evacs finish
                    for ch in range(NB):
                        nc.sync.dma_start(
                            codes_dram.ap()[img].rearrange("(b p) c -> p b c", b=NB)[:, ch:ch + 1, :],
                            code8[:, ch * W:(ch + 1) * W].rearrange("p (b c) -> p b c", b=1))
                else:
                    nc.sync.dma_start(codes_dram.ap()[img].rearrange("(b p) c -> p b c", b=NB),
                                      code8[:].rearrange("p (b c) -> p b c", b=NB))
    nc.compile()
    return nc


def _get_nc():
    if "nc" not in _CACHE:
        _CACHE["nc"] = _build_nc()
    return _CACHE["nc"]


# reference-frame neighbor offsets into the ((3,3),(5,5))-padded image
_NB_OFF = [(0, 5, 1), (0, 8, 2), (3, 10, 4), (6, 8, 8),
           (6, 5, 16), (6, 2, 32), (3, 0, 64), (0, 2, 128)]

# device-unresolvable rows: top 3 of each 128-row block (backward row shift
# cannot cross blocks) and the last 3 rows (stale up3 tail)
_PATCH_ROWS = np.array(sorted({128 * b + r for b in range(NB) for r in range(3)} |
                              {509, 510, 511}), dtype=np.int64)
# device-unresolvable cols: backward col shifts read zeroed mask halos
_PATCH_COLS = np.array([0, 1, 2, 3, 4, 509, 510, 511], dtype=np.int64)


def kernel(x: np.ndarray) -> np.ndarray:
    x = np.ascontiguousarray(x, dtype=np.float32)
    nc = _get_nc()
    wm = _weight_mats()
    in_maps = [{"x": x[c * NIMG:(c + 1) * NIMG], "wmat": wm} for c in range(8)]
    res = run_bass_kernel_spmd(nc, in_maps, list(range(8)))
    codes = np.concatenate([res.results[c]["codes"] for c in range(8)],
                           axis=0).astype(np.int32)      # [128, H, W]

    B = codes.shape[0]
    c_true = (x * 255.0).astype(np.uint8).astype(np.int32)
    pad = np.pad(c_true, ((0, 0), (3, 3), (5, 5)))

    # pixels needing exact host recompute
    aff = np.zeros((B, H, W), dtype=bool)
    aff[:, _PATCH_ROWS, :] = True
    aff[:, :, _PATCH_COLS] = True
    # ties: device's backward bit misses [im(P-Delta) == im(P)]
    affp = np.zeros((B, H + 6, W + 10), dtype=bool)
    for dy, dx, _, _ in PAIRS:
        t = pad[:, 3 + dy:3 + dy + H, 5 + dx:5 + dx + W] == c_true  # tie at Q
        affp[:, 3 + dy:3 + dy + H, 5 + dx:5 + dx + W] |= t          # affects Q+Delta
    aff |= affp[:, 3:3 + H, 5:5 + W]
    # RNE(v-0.5) vs floor(v) divergence on device quantization
    v = x * np.float32(255.0)
    bad = np.rint(v - np.float32(0.5)).astype(np.int32) != c_true
    if bad.any():
        aff |= bad
        badp = np.pad(bad, ((0, 0), (3, 3), (5, 5)))
        for dy, dx, _ in _NB_OFF:
            aff |= badp[:, dy:dy + H, dx:dx + W]

    bi, yi, xi = np.nonzero(aff)
    center = c_true[bi, yi, xi]
    z = np.zeros_like(center)
    for dy, dx, w in _NB_OFF:
        z = z + (pad[bi, yi + dy, xi + dx] >= center).astype(np.int32) * w
    codes[bi, yi, xi] = z

    seg = (codes + 256 * np.arange(B, dtype=np.int32)[:, None, None]).ravel()
    hist = np.bincount(seg, minlength=B * 256).reshape(B, 256)
    uni = hist[:, UNIS]                                   # [128, 58]
    rest = hist.sum(-1, keepdims=True) - uni.sum(-1, keepdims=True)
    out = np.concatenate([uni, rest], axis=-1)
    return np.mod(out, 256).astype(np.float32)            # [128, 59]


# revision 4
# speedup vs baseline: 1.0067x; 1.0067x over previous
"""LBP-5x3 code kernel for TRN2 (8 NeuronCores, data parallel) + host binning.

Full inputs: x [128, 512, 512] fp32 in [0,1). Output: [128, 59] fp32.
Each core processes 16 images laid out [128 partitions = rows-in-block,
4 row-blocks x (5+512+5) haloed cols].

The 8 LBP neighbors form 4 symmetric offset pairs +-Delta. For each pair the
device computes ONE non-strict mask m(P) = [im(P+Delta) >= im(P)] (DVE is_ge;
4 passes instead of 8). The backward bit is recovered linearly:
  m_bwd(P) = [im(P-Delta) >= im(P)] = 1 - m(P-Delta) + [im(P-Delta)==im(P)]
The PE reads each mask tile twice - once aligned with weight +w_fwd, once
shifted by Delta with weight -w_bwd (row shift folded into a shifted-identity
lhsT, col shift into the rhs column offset) - and the PSUM->SBUF evacuation
adds the constant sum(w_bwd)=195. The tie term and all border rows/cols are
patched exactly on the host (~1-2% of pixels), as are RNE-vs-floor pixels of
the device's uint8 quantization.

Per image on device:
  ACT:  ci = RNE(x*255 - 0.5) -> int16, haloed cols (halos zeroed, Pool)
  DMA:  up3[p] = ci[p+3] (one SBUF-SBUF row-shift + block wrap)
  DVE:  4 is_ge masks -> bf16, haloed mask tiles (halos zeroed once)
  PE:   7 matmuls per 512-col chunk (3 pairs fwd+bwd, dx=0 pair fused)
  ACT:  PSUM + 195 -> uint8 codes, DMA out
Host: patch rows {128b+0..2, 509..511}, cols {0..4, 509..511}, tie pixels,
RNE pixels; per-image 256-bincount -> 58 uniform bins + catch-all, mod 256.
"""
import sys

sys.path.insert(0, "/opt/trn_rl_repo")
sys.path.insert(0, "/opt/pypackages")

import numpy as np
import ml_dtypes

import concourse.bacc as bacc
import concourse.tile as tile
from concourse import mybir
from concourse.bass_utils import run_bass_kernel_spmd

UNIS = np.array([0, 1, 2, 3, 4, 6, 7, 8, 12, 14, 15, 16, 24, 28, 30, 31, 32, 48, 56,
                 60, 62, 63, 64, 96, 112, 120, 124, 126, 127, 128, 129, 131, 135, 143,
                 159, 191, 192, 193, 195, 199, 207, 223, 224, 225, 227, 231, 239, 240,
                 241, 243, 247, 248, 249, 251, 252, 253, 254, 255], dtype=np.int32)

# 4 symmetric pairs: (dy, dx) forward offset, forward weight, backward weight
PAIRS = [(3, 3, 8, 128), (3, 0, 16, 1), (3, -3, 32, 2), (0, 5, 4, 64)]
WSUM_BWD = float(sum(wb for (_, _, _, wb) in PAIRS))  # 195

NIMG = 16          # images per core
H = W = 512
NB = 4             # row blocks of 128
OFFC = 5           # halo width (image col offset inside a block)
BW = W + 2 * OFFC  # block width with halo (522)
FW = NB * BW       # full free width of haloed tiles (2088)
CW = NB * W        # full free width of compact tiles (2048)

F32 = mybir.dt.float32
BF16 = mybir.dt.bfloat16
I16 = mybir.dt.int16
U8 = mybir.dt.uint8

_CACHE = {}


def _hv(t, start, width=W):
    """3D AP over a haloed [128, FW] tile: 4 blocks x width cols from `start`."""
    return t[:].rearrange("p (b c) -> p b c", b=NB)[:, :, start:start + width]


def _weight_mats() -> np.ndarray:
    """lhsT weight matrices [7, 128, 128] bf16: out[p,c] = sum_q W[q,p] m[q,c].
    Aligned read: W = w*I. Row-shifted read (mask lives at q = r-3):
    W[q, p] = w * delta(q == p-3). The dx=0 pair fuses fwd+bwd into one W."""
    eye = np.eye(128, dtype=np.float32)
    shift = np.zeros((128, 128), dtype=np.float32)
    shift[np.arange(125), np.arange(3, 128)] = 1.0  # S[q, p] = 1 iff q == p-3
    mats = [
        16.0 * eye - 1.0 * shift,   # pair (3,0): fused fwd 16I, bwd -1*S
        8.0 * eye,                  # pair (3,3) fwd
        -128.0 * shift,             # pair (3,3) bwd
        32.0 * eye,                 # pair (3,-3) fwd
        -2.0 * shift,               # pair (3,-3) bwd
        4.0 * eye,                  # pair (0,5) fwd
        -64.0 * eye,                # pair (0,5) bwd
    ]
    return np.stack(mats).astype(ml_dtypes.bfloat16)


def _build_nc():
    nc = bacc.Bacc("TRN2", target_bir_lowering=False, debug=False, num_devices=8)
    x = nc.dram_tensor("x", [NIMG, H, W], F32, kind="ExternalInput")
    wmat = nc.dram_tensor("wmat", [7, 128, 128], BF16, kind="ExternalInput")
    codes_dram = nc.dram_tensor("codes", [NIMG, H, W], U8, kind="ExternalOutput")

    with tile.TileContext(nc) as tc:
        with tc.tile_pool(name="pc", bufs=1) as poolc, \
                tc.tile_pool(name="px", bufs=5) as poolx, \
                tc.tile_pool(name="pm", bufs=2) as poolm, \
                tc.tile_pool(name="ps", bufs=8, space="PSUM") as poolp:
            wt = poolc.tile([128, 7 * 128], BF16, name="wt", tag="wt")
            # (wmat DMA is emitted after image 0's first block load)

            def w_ap(k):
                return wt[:, k * 128:(k + 1) * 128]

            for img in range(NIMG):
                xf = poolx.tile([128, CW], F32, name="xf", tag="xf")
                ci = poolx.tile([128, FW], I16, name="ci", tag="ci")
                nc.gpsimd.memset(ci[:].rearrange("p (b c) -> p b c", b=NB)[:, :, 0:OFFC], 0.0)
                nc.gpsimd.memset(ci[:].rearrange("p (b c) -> p b c", b=NB)[:, :, OFFC + W:BW], 0.0)
                if img == 0:
                    # block-granular prologue: lets the first compares (and so
                    # the PE) start ~3 DMA+convert legs earlier
                    for b in range(NB):
                        nc.sync.dma_start(
                            xf[:].rearrange("p (b c) -> p b c", b=NB)[:, b:b + 1, :],
                            x.ap()[img].rearrange("(b p) c -> p b c", b=NB)[:, b:b + 1, :])
                        if b == 0:
                            nc.sync.dma_start(wt[:].rearrange("p (k q) -> p k q", k=7),
                                              wmat.ap().rearrange("k p q -> p k q"))
                        nc.scalar.activation(
                            out=_hv(ci, OFFC)[:, b:b + 1, :],
                            in_=xf[:].rearrange("p (b c) -> p b c", b=NB)[:, b:b + 1, :],
                            func=mybir.ActivationFunctionType.Copy,
                            bias=-0.5, scale=255.0)
                else:
                    nc.sync.dma_start(xf[:].rearrange("p (b c) -> p b c", b=NB),
                                      x.ap()[img].rearrange("(b p) c -> p b c", b=NB))
                    nc.scalar.activation(out=_hv(ci, OFFC),
                                         in_=xf[:].rearrange("p (b c) -> p b c", b=NB),
                                         func=mybir.ActivationFunctionType.Copy,
                                         bias=-0.5, scale=255.0)
                # up3[p] = ci[p+3]: rows shifted up by 3 within each block,
                # block wrap for the last 3 partitions. The tail (rows past
                # 511) stays stale (finite int16); those rows are host-patched.
                up3 = poolx.tile([128, FW], I16, name="up3", tag="up3")
                if img == 0:
                    for b in range(NB):
                        nc.sync.dma_start(up3[0:125, b * BW:(b + 1) * BW],
                                          ci[3:128, b * BW:(b + 1) * BW])
                        if b > 0:
                            nc.sync.dma_start(up3[125:128, (b - 1) * BW:b * BW],
                                              ci[0:3, b * BW:(b + 1) * BW])
                else:
                    nc.sync.dma_start(up3[0:125, :], ci[3:128, :])
                    nc.sync.dma_start(up3[125:128, 0:FW - BW].rearrange("p (b c) -> p b c", b=NB - 1),
                                      ci[0:3, BW:FW].rearrange("p (b c) -> p b c", b=NB - 1))

                # 4 non-strict masks, haloed tiles (halos zeroed on the first
                # two iterations = both pool buffers, never written again).
                # The (0,5) mask goes first: it needs only ci, so the PE can
                # start before the up3 shift DMA lands.
                masks = {}
                for pi, (dy, dx, _, _) in sorted(enumerate(PAIRS),
                                                 key=lambda e: e[1][0]):
                    mt = poolm.tile([128, FW], BF16, name=f"m{pi}", tag=f"m{pi}")
                    if img < 2:
                        nc.gpsimd.memset(mt[:].rearrange("p (b c) -> p b c", b=NB)[:, :, 0:OFFC], 0.0)
                        nc.gpsimd.memset(mt[:].rearrange("p (b c) -> p b c", b=NB)[:, :, OFFC + W:BW], 0.0)
                    src = up3 if dy == 3 else ci
                    if img == 0:
                        # per-block, so each PE chunk can start off block ch
                        for b in range(NB):
                            nc.vector.tensor_tensor(
                                out=_hv(mt, OFFC)[:, b:b + 1, :],
                                in0=_hv(src, OFFC + dx)[:, b:b + 1, :],
                                in1=_hv(ci, OFFC)[:, b:b + 1, :],
                                op=mybir.AluOpType.is_ge)
                    else:
                        nc.vector.tensor_tensor(out=_hv(mt, OFFC),
                                                in0=_hv(src, OFFC + dx),
                                                in1=_hv(ci, OFFC),
                                                op=mybir.AluOpType.is_ge)
                    masks[pi] = mt

                # 7 matmuls per 512-col chunk: (lhsT index, mask, col offset),
                # ordered by mask readiness
                mm = [(5, masks[3], OFFC),        # (0,5) fwd +4I
                      (6, masks[3], OFFC - 5),    # (0,5) bwd -64I, cols c-5
                      (1, masks[0], OFFC),        # (3,3) fwd +8I
                      (2, masks[0], OFFC - 3),    # (3,3) bwd -128S, cols c-3
                      (0, masks[1], OFFC),        # (3,0) fused 16I - S
                      (3, masks[2], OFFC),        # (3,-3) fwd +32I
                      (4, masks[2], OFFC + 3)]    # (3,-3) bwd -2S, cols c+3
                code8 = poolx.tile([128, CW], U8, name="code8", tag="code8")
                for ch in range(NB):
                    cps = poolp.tile([128, W], F32, name="cps", tag="cps")
                    for i, (k, mt, co) in enumerate(mm):
                        rhs = mt[:, ch * BW + co: ch * BW + co + W]
                        nc.tensor.matmul(out=cps[:], lhsT=w_ap(k), rhs=rhs,
                                         start=(i == 0), stop=(i == len(mm) - 1))
                    # code = psum + sum(w_bwd); exact ints, safe in [0, 255]
                    nc.scalar.activation(out=code8[:, ch * W:(ch + 1) * W], in_=cps[:],
                                         func=mybir.ActivationFunctionType.Copy,
                                         bias=WSUM_BWD, scale=1.0)
                if img == NIMG - 1:
                    # drain faster: ship chunks as their evacs finish
                    for ch in range(NB):
                        nc.sync.dma_start(
                            codes_dram.ap()[img].rearrange("(b p) c -> p b c", b=NB)[:, ch:ch + 1, :],
                            code8[:, ch * W:(ch + 1) * W].rearrange("p (b c) -> p b c", b=1))
                else:
                    nc.sync.dma_start(codes_dram.ap()[img].rearrange("(b p) c -> p b c", b=NB),
                                        code8[:].rearrange("p (b c) -> p b c", b=NB))
    nc.compile()
    return nc


def _get_nc():
    if "nc" not in _CACHE:
        _CACHE["nc"] = _build_nc()
    return _CACHE["nc"]


# reference-frame neighbor offsets into the ((3,3),(5,5))-padded image
_NB_OFF = [(0, 5, 1), (0, 8, 2), (3, 10, 4), (6, 8, 8),
           (6, 5, 16), (6, 2, 32), (3, 0, 64), (0, 2, 128)]

# device-unresolvable rows: top 3 of each 128-row block (backward row shift
# cannot cross blocks) and the last 3 rows (stale up3 tail)
_PATCH_ROWS = np.array(sorted({128 * b + r for b in range(NB) for r in range(3)} |
                              {509, 510, 511}), dtype=np.int64)
# device-unresolvable cols: backward col shifts read zeroed mask halos
_PATCH_COLS = np.array([0, 1, 2, 3, 4, 509, 510, 511], dtype=np.int64)


def kernel(x: np.ndarray) -> np.ndarray:
    x = np.ascontiguousarray(x, dtype=np.float32)
    nc = _get_nc()
    wm = _weight_mats()
    in_maps = [{"x": x[c * NIMG:(c + 1) * NIMG], "wmat": wm} for c in range(8)]
    res = run_bass_kernel_spmd(nc, in_maps, list(range(8)))
    codes = np.concatenate([res.results[c]["codes"] for c in range(8)],
                           axis=0).astype(np.int32)      # [128, H, W]

    B = codes.shape[0]
    c_true = (x * 255.0).astype(np.uint8).astype(np.int32)
    pad = np.pad(c_true, ((0, 0), (3, 3), (5, 5)))

    # pixels needing exact host recompute
    aff = np.zeros((B, H, W), dtype=bool)
    aff[:, _PATCH_ROWS, :] = True
    aff[:, :, _PATCH_COLS] = True
    # ties: device's backward bit misses [im(P-Delta) == im(P)]
    affp = np.zeros((B, H + 6, W + 10), dtype=bool)
    for dy, dx, _, _ in PAIRS:
        t = pad[:, 3 + dy:3 + dy + H, 5 + dx:5 + dx + W] == c_true  # tie at Q
        affp[:, 3 + dy:3 + dy + H, 5 + dx:5 + dx + W] |= t          # affects Q+Delta
    aff |= affp[:, 3:3 + H, 5:5 + W]
    # RNE(v-0.5) vs floor(v) divergence on device quantization
    v = x * np.float32(255.0)
    bad = np.rint(v - np.float32(0.5)).astype(np.int32) != c_true
    if bad.any():
        aff |= bad
        badp = np.pad(bad, ((0, 0), (3, 3), (5, 5)))
        for dy, dx, _ in _NB_OFF:
            aff |= badp[:, dy:dy + H, dx:dx + W]

    bi, yi, xi = np.nonzero(aff)
    center = c_true[bi, yi, xi]
    z = np.zeros_like(center)
    for dy, dx, w in _NB_OFF:
        z = z + (pad[bi, yi + dy, xi + dx] >= center).astype(np.int32) * w
    codes[bi, yi, xi] = z

    seg = (codes + 256 * np.arange(B, dtype=np.int32)[:, None, None]).ravel()
    hist = np.bincount(seg, minlength=B * 256).reshape(B, 256)
    uni = hist[:, UNIS]                                   # [128, 58]
    rest = hist.sum(-1, keepdims=True) - uni.sum(-1, keepdims=True)
    out = np.concatenate([uni, rest], axis=-1)
    return np.mod(out, 256).astype(np.float32)            # [128, 59]


# revision 5
# speedup vs baseline: 1.0247x; 1.0179x over previous
"""LBP-5x3 code kernel for TRN2 (8 NeuronCores, data parallel) + host binning.

Full inputs: x [128, 512, 512] fp32 in [0,1). Output: [128, 59] fp32.
Each core processes 16 images laid out [128 partitions = rows-in-block,
4 row-blocks x (5+512+5) haloed cols].

The 8 LBP neighbors form 4 symmetric offset pairs +-Delta. For each pair the
device computes ONE non-strict mask m(P) = [im(P+Delta) >= im(P)] (DVE is_ge;
4 passes instead of 8). The backward bit is recovered linearly:
  m_bwd(P) = [im(P-Delta) >= im(P)] = 1 - m(P-Delta) + [im(P-Delta)==im(P)]
The PE reads each mask tile twice - once aligned with weight +w_fwd, once
shifted by Delta with weight -w_bwd (row shift folded into a shifted-identity
lhsT, col shift into the rhs column offset) - and the PSUM->SBUF evacuation
adds the constant sum(w_bwd)=195. The tie term and all border rows/cols are
patched exactly on the host (~1-2% of pixels), as are RNE-vs-floor pixels of
the device's uint8 quantization.

Per image on device:
  ACT:  ci = RNE(x*255 - 0.5) -> int16, haloed cols (halos zeroed, Pool)
  DMA:  up3[p] = ci[p+3] (one SBUF-SBUF row-shift + block wrap)
  DVE:  4 is_ge masks -> bf16, haloed mask tiles (halos zeroed once)
  PE:   7 matmuls per 512-col chunk (3 pairs fwd+bwd, dx=0 pair fused)
  ACT:  PSUM + 195 -> uint8 codes, DMA out
Host: patch rows {128b+0..2, 509..511}, cols {0..4, 509..511}, tie pixels,
RNE pixels; per-image 256-bincount -> 58 uniform bins + catch-all, mod 256.
"""
import sys

sys.path.insert(0, "/opt/trn_rl_repo")
sys.path.insert(0, "/opt/pypackages")

import numpy as np

import concourse.bacc as bacc
import concourse.tile as tile
from concourse import mybir
from concourse.bass_utils import run_bass_kernel_spmd
from concourse.masks import make_identity

UNIS = np.array([0, 1, 2, 3, 4, 6, 7, 8, 12, 14, 15, 16, 24, 28, 30, 31, 32, 48, 56,
                 60, 62, 63, 64, 96, 112, 120, 124, 126, 127, 128, 129, 131, 135, 143,
                 159, 191, 192, 193, 195, 199, 207, 223, 224, 225, 227, 231, 239, 240,
                 241, 243, 247, 248, 249, 251, 252, 253, 254, 255], dtype=np.int32)

# 4 symmetric pairs: (dy, dx) forward offset, forward weight, backward weight
PAIRS = [(3, 3, 8, 128), (3, 0, 16, 1), (3, -3, 32, 2), (0, 5, 4, 64)]
WSUM_BWD = float(sum(wb for (_, _, _, wb) in PAIRS))  # 195

NIMG = 16          # images per core
H = W = 512
NB = 4             # row blocks of 128
OFFC = 5           # halo width (image col offset inside a block)
BW = W + 2 * OFFC  # block width with halo (522)
FW = NB * BW       # full free width of haloed tiles (2088)
CW = NB * W        # full free width of compact tiles (2048)

F32 = mybir.dt.float32
BF16 = mybir.dt.bfloat16
I16 = mybir.dt.int16
U8 = mybir.dt.uint8

_CACHE = {}


def _hv(t, start, width=W):
    """3D AP over a haloed [128, FW] tile: 4 blocks x width cols from `start`."""
    return t[:].rearrange("p (b c) -> p b c", b=NB)[:, :, start:start + width]


def _build_nc():
    nc = bacc.Bacc("TRN2", target_bir_lowering=False, debug=False, num_devices=8)
    x = nc.dram_tensor("x", [NIMG, H, W], F32, kind="ExternalInput")
    codes_dram = nc.dram_tensor("codes", [NIMG, H, W], U8, kind="ExternalOutput")

    with tile.TileContext(nc) as tc:
        with tc.tile_pool(name="pc", bufs=1) as poolc, \
                tc.tile_pool(name="px", bufs=5) as poolx, \
                tc.tile_pool(name="pm", bufs=2) as poolm, \
                tc.tile_pool(name="ps", bufs=8, space="PSUM") as poolp:
            # lhsT weight matrices [128, 7*128] bf16, built on-chip at t=0 on
            # otherwise-idle engines (cheaper than a DMA in the fill window):
            # out[p,c] = sum_q W[q,p] m[q,c]. Aligned read: W = w*I.
            # Row-shifted read (mask lives at q = r-3): W[q,p] = w*d(q==p-3),
            # i.e. cols 3: of W hold w*I cols 0:125. The dx=0 pair fuses
            # fwd+bwd into W0 = 16I - S.
            wt = poolc.tile([128, 7 * 128], BF16, name="wt", tag="wt")
            ident = poolc.tile([128, 128], F32, name="ident", tag="ident")
            tmp16 = poolc.tile([128, 128], BF16, name="tmp16", tag="tmp16")
            sbase = poolc.tile([128, 128], BF16, name="sbase", tag="sbase")
            make_identity(nc, ident[:])

            def w_ap(k):
                return wt[:, k * 128:(k + 1) * 128]

            for k, w in ((1, 8.0), (3, 32.0), (5, 4.0), (6, -64.0)):
                nc.scalar.mul(w_ap(k), ident[:], w)
            for k, w in ((2, -128.0), (4, -2.0)):
                nc.vector.memset(w_ap(k)[:, 0:3], 0.0)
                nc.scalar.mul(w_ap(k)[:, 3:128], ident[:, 0:125], w)
            nc.scalar.mul(tmp16[:], ident[:], 16.0)
            nc.vector.memset(sbase[:, 0:3], 0.0)
            nc.scalar.mul(sbase[:, 3:128], ident[:, 0:125], 1.0)
            nc.vector.tensor_tensor(out=w_ap(0), in0=tmp16[:], in1=sbase[:],
                                    op=mybir.AluOpType.subtract)

            for img in range(NIMG):
                xf = poolx.tile([128, CW], F32, name="xf", tag="xf")
                ci = poolx.tile([128, FW], I16, name="ci", tag="ci")
                nc.gpsimd.memset(ci[:].rearrange("p (b c) -> p b c", b=NB)[:, :, 0:OFFC], 0.0)
                nc.gpsimd.memset(ci[:].rearrange("p (b c) -> p b c", b=NB)[:, :, OFFC + W:BW], 0.0)
                if img == 0:
                    # block-granular prologue: lets the first compares (and so
                    # the PE) start ~3 DMA+convert legs earlier
                    for b in range(NB):
                        nc.sync.dma_start(
                            xf[:].rearrange("p (b c) -> p b c", b=NB)[:, b:b + 1, :],
                            x.ap()[img].rearrange("(b p) c -> p b c", b=NB)[:, b:b + 1, :])
                        nc.scalar.activation(
                            out=_hv(ci, OFFC)[:, b:b + 1, :],
                            in_=xf[:].rearrange("p (b c) -> p b c", b=NB)[:, b:b + 1, :],
                            func=mybir.ActivationFunctionType.Copy,
                            bias=-0.5, scale=255.0)
                else:
                    nc.sync.dma_start(xf[:].rearrange("p (b c) -> p b c", b=NB),
                                      x.ap()[img].rearrange("(b p) c -> p b c", b=NB))
                    nc.scalar.activation(out=_hv(ci, OFFC),
                                         in_=xf[:].rearrange("p (b c) -> p b c", b=NB),
                                         func=mybir.ActivationFunctionType.Copy,
                                         bias=-0.5, scale=255.0)
                # up3[p] = ci[p+3]: rows shifted up by 3 within each block,
                # block wrap for the last 3 partitions. The tail (rows past
                # 511) stays stale (finite int16); those rows are host-patched.
                up3 = poolx.tile([128, FW], I16, name="up3", tag="up3")
                if img == 0:
                    for b in range(NB):
                        nc.sync.dma_start(up3[0:125, b * BW:(b + 1) * BW],
                                          ci[3:128, b * BW:(b + 1) * BW])
                        if b > 0:
                            nc.sync.dma_start(up3[125:128, (b - 1) * BW:b * BW],
                                              ci[0:3, b * BW:(b + 1) * BW])
                else:
                    nc.sync.dma_start(up3[0:125, :], ci[3:128, :])
                    nc.sync.dma_start(up3[125:128, 0:FW - BW].rearrange("p (b c) -> p b c", b=NB - 1),
                                      ci[0:3, BW:FW].rearrange("p (b c) -> p b c", b=NB - 1))

                # 4 non-strict masks, haloed tiles (halos zeroed on the first
                # two iterations = both pool buffers, never written again).
                # The (0,5) mask goes first: it needs only ci, so the PE can
                # start before the up3 shift DMA lands.
                masks = {}
                for pi, (dy, dx, _, _) in sorted(enumerate(PAIRS),
                                                 key=lambda e: e[1][0]):
                    mt = poolm.tile([128, FW], BF16, name=f"m{pi}", tag=f"m{pi}")
                    if img < 2:
                        nc.gpsimd.memset(mt[:].rearrange("p (b c) -> p b c", b=NB)[:, :, 0:OFFC], 0.0)
                        nc.gpsimd.memset(mt[:].rearrange("p (b c) -> p b c", b=NB)[:, :, OFFC + W:BW], 0.0)
                    src = up3 if dy == 3 else ci
                    if img == 0:
                        # per-block, so each PE chunk can start off block ch
                        for b in range(NB):
                            nc.vector.tensor_tensor(
                                out=_hv(mt, OFFC)[:, b:b + 1, :],
                                in0=_hv(src, OFFC + dx)[:, b:b + 1, :],
                                in1=_hv(ci, OFFC)[:, b:b + 1, :],
                                op=mybir.AluOpType.is_ge)
                    else:
                        nc.vector.tensor_tensor(out=_hv(mt, OFFC),
                                                in0=_hv(src, OFFC + dx),
                                                in1=_hv(ci, OFFC),
                                                op=mybir.AluOpType.is_ge)
                    masks[pi] = mt

                # 7 matmuls per 512-col chunk: (lhsT index, mask, col offset),
                # ordered by mask readiness
                mm = [(5, masks[3], OFFC),        # (0,5) fwd +4I
                      (6, masks[3], OFFC - 5),    # (0,5) bwd -64I, cols c-5
                      (1, masks[0], OFFC),        # (3,3) fwd +8I
                      (2, masks[0], OFFC - 3),    # (3,3) bwd -128S, cols c-3
                      (0, masks[1], OFFC),        # (3,0) fused 16I - S
                      (3, masks[2], OFFC),        # (3,-3) fwd +32I
                      (4, masks[2], OFFC + 3)]    # (3,-3) bwd -2S, cols c+3
                code8 = poolx.tile([128, CW], U8, name="code8", tag="code8")
                for ch in range(NB):
                    cps = poolp.tile([128, W], F32, name="cps", tag="cps")
                    for i, (k, mt, co) in enumerate(mm):
                        rhs = mt[:, ch * BW + co: ch * BW + co + W]
                        nc.tensor.matmul(out=cps[:], lhsT=w_ap(k), rhs=rhs,
                                         start=(i == 0), stop=(i == len(mm) - 1))
                    # code = psum + sum(w_bwd); exact ints, safe in [0, 255]
                    nc.scalar.activation(out=code8[:, ch * W:(ch + 1) * W], in_=cps[:],
                                         func=mybir.ActivationFunctionType.Copy,
                                         bias=WSUM_BWD, scale=1.0)
                if img == NIMG - 1:
                    # drain faster: ship chunks as their evacs finish
                    for ch in range(NB):
                        nc.sync.dma_start(
                            codes_dram.ap()[img].rearrange("(b p) c -> p b c", b=NB)[:, ch:ch + 1, :],
                            code8[:, ch * W:(ch + 1) * W].rearrange("p (b c) -> p b c", b=1))
                else:
                    nc.sync.dma_start(codes_dram.ap()[img].rearrange("(b p) c -> p b c", b=NB),
                                        code8[:].rearrange("p (b c) -> p b c", b=NB))
    nc.compile()
    return nc


def _get_nc():
    if "nc" not in _CACHE:
        _CACHE["nc"] = _build_nc()
    return _CACHE["nc"]


# reference-frame neighbor offsets into the ((3,3),(5,5))-padded image
_NB_OFF = [(0, 5, 1), (0, 8, 2), (3, 10, 4), (6, 8, 8),
           (6, 5, 16), (6, 2, 32), (3, 0, 64), (0, 2, 128)]

# device-unresolvable rows: top 3 of each 128-row block (backward row shift
# cannot cross blocks) and the last 3 rows (stale up3 tail)
_PATCH_ROWS = np.array(sorted({128 * b + r for b in range(NB) for r in range(3)} |
                              {509, 510, 511}), dtype=np.int64)
# device-unresolvable cols: backward col shifts read zeroed mask halos
_PATCH_COLS = np.array([0, 1, 2, 3, 4, 509, 510, 511], dtype=np.int64)


def kernel(x: np.ndarray) -> np.ndarray:
    x = np.ascontiguousarray(x, dtype=np.float32)
    nc = _get_nc()
    in_maps = [{"x": x[c * NIMG:(c + 1) * NIMG]} for c in range(8)]
    res = run_bass_kernel_spmd(nc, in_maps, list(range(8)))
    codes = np.concatenate([res.results[c]["codes"] for c in range(8)],
                           axis=0).astype(np.int32)      # [128, H, W]

    B = codes.shape[0]
    c_true = (x * 255.0).astype(np.uint8).astype(np.int32)
    pad = np.pad(c_true, ((0, 0), (3, 3), (5, 5)))

    # pixels needing exact host recompute
    aff = np.zeros((B, H, W), dtype=bool)
    aff[:, _PATCH_ROWS, :] = True
    aff[:, :, _PATCH_COLS] = True
    # ties: device's backward bit misses [im(P-Delta) == im(P)]
    affp = np.zeros((B, H + 6, W + 10), dtype=bool)
    for dy, dx, _, _ in PAIRS:
        t = pad[:, 3 + dy:3 + dy + H, 5 + dx:5 + dx + W] == c_true  # tie at Q
        affp[:, 3 + dy:3 + dy + H, 5 + dx:5 + dx + W] |= t          # affects Q+Delta
    aff |= affp[:, 3:3 + H, 5:5 + W]
    # RNE(v-0.5) vs floor(v) divergence on device quantization
    v = x * np.float32(255.0)
    bad = np.rint(v - np.float32(0.5)).astype(np.int32) != c_true
    if bad.any():
        aff |= bad
        badp = np.pad(bad, ((0, 0), (3, 3), (5, 5)))
        for dy, dx, _ in _NB_OFF:
            aff |= badp[:, dy:dy + H, dx:dx + W]

    bi, yi, xi = np.nonzero(aff)
    center = c_true[bi, yi, xi]
    z = np.zeros_like(center)
    for dy, dx, w in _NB_OFF:
        z = z + (pad[bi, yi + dy, xi + dx] >= center).astype(np.int32) * w
    codes[bi, yi, xi] = z

    seg = (codes + 256 * np.arange(B, dtype=np.int32)[:, None, None]).ravel()
    hist = np.bincount(seg, minlength=B * 256).reshape(B, 256)
    uni = hist[:, UNIS]                                   # [128, 58]
    rest = hist.sum(-1, keepdims=True) - uni.sum(-1, keepdims=True)
    out = np.concatenate([uni, rest], axis=-1)
    return np.mod(out, 256).astype(np.float32)            # [128, 59]
